# revision 2
# baseline (speedup 1.0000x reference)
"""Trainium2 Bass kernel v2 for the dense transformer block (pre-LN MHA + MLP).

Data parallel across 8 cores (batch -> core). Per core [1024, 1024]:

- fp8e4 DoubleRow matmuls everywhere the contraction allows:
  qkv / proj (k-chunk pairs), scores (head-dim split 2x32 across slot dim,
  32-partition stationary), pav (key-chunk pairs), fc1/fc2 (weight hi+lo
  fp8 split in the slot dim, activation streamed to both slots with a
  0-stride AP).
- Weights scaled x1024 on host (fp8 subnormal escape); descale folded into
  epilogues; x1 kept scaled by 1024 (LN is scale-invariant) so the residual
  adds are PE identity-matmuls and the only descale is the final y copy.
- Softmax exp split across ACT (hw exp) and DVE/GPSIMD (Schraudolph bit
  trick, fine since softmax normalizes away small relative error).
- Attention + MLP processed in query halves so the second half's exp wall
  overlaps the first half's MLP matmuls.
"""

from contextlib import ExitStack

import numpy as np

import concourse.bass as bass
import concourse.tile as tile
from concourse import bacc, mybir
from concourse.bass import ts
from concourse.bass_utils import run_bass_kernel_spmd
from concourse.masks import make_identity

F32 = mybir.dt.float32
F16 = mybir.dt.float16
F8 = mybir.dt.float8e4
I32 = mybir.dt.int32
AF = mybir.ActivationFunctionType
ALU = mybir.AluOpType
DR = mybir.MatmulPerfMode.DoubleRow

P = 128
N = 1024         # tokens per core
D = 1024
KC = 8           # contraction chunks of 128 over D
HEADS = 16
HD = 64
HID = 4096
EPS = 1e-6
NT = 2
MT = 8
QH = 512         # query half
SCALE = HD ** -0.5
SW = 1024.0      # weight scale (fp8)
RSW = 1.0 / SW
VP = 65          # v_sb per-head pitch: 64 dims + ones col (mk step 16*65)

# Schraudolph exp: exp(SCALE*x) ~ bitcast_f32(int32(EA*x + EB))
EA = (2 ** 23) / np.log(2.0) * SCALE
EB = 127.0 * (2 ** 23) - 486411.0

# exp engine split per (mk index 0..7): A=ACT hw exp, D=DVE+GPSIMD split
EXP_PAT_A = ["A", "D", "A", "A", "A", "D", "A", "A"]  # phase A (no mlp rival)
EXP_PAT_B = ["A", "D", "A", "A", "D", "A", "A", "D"]  # phase B (gelu on ACT)
PULL_MOD = 4       # pull a thunk after every PULL_MOD-th mk in scores_head
USE_PM_B = 99      # attnB heads >= this borrow pm psum slots for scores
PROBS_BUFS = 2
XLOAD_BUFS = 2


def slot0(ap_, n):
    """[p, 2(0-stride), n] view of a [p, n] AP: stream same data to both
    DoubleRow slots."""
    return bass.AP(
        tensor=ap_.tensor, offset=ap_.offset,
        ap=[list(ap_.ap[0]), [0, 2], [ap_.ap[-1][0], n]],
    )


def scaled_copy(nc, ei, dst, src, s):
    """dst = src * s on DVE (even) or ACT (odd). GPSIMD cannot read PSUM."""
    if ei % 2 == 0:
        nc.vector.tensor_scalar_mul(dst, src, s)
    else:
        nc.scalar.activation(dst, src, AF.Copy, scale=s)


def eng_copy(nc, ei, dst, src):
    if ei % 2 == 0:
        nc.vector.tensor_copy(dst, src)
    else:
        nc.scalar.copy(dst, src)


def build_block(ln1_triv, ln2_triv, qk_triv, apply_c1, apply_bfc2,
                fc1b_triv=True, stop_after=None, phase_marks=None):
    nc = bacc.Bacc("TRN2", target_bir_lowering=False, debug=False, num_devices=8)

    class _Stop(Exception):
        pass

    def mark(name):
        if phase_marks is not None:
            phase_marks.append((name, len(nc.m.functions[0].blocks[0].instructions)))
        if stop_after is not None and name == stop_after:
            raise _Stop()

    x_d = nc.dram_tensor("x", [N, D], F32, kind="ExternalInput")
    wqkv_d = nc.dram_tensor("w_qkv", [D, 3 * D], F8, kind="ExternalInput")
    wproj_d = nc.dram_tensor("w_proj", [D, D], F8, kind="ExternalInput")
    # piece-contiguous: [piece, p, (kc two col)] so each stream is a 2D DMA
    wfc1_d = nc.dram_tensor("w_fc1", [8, P, KC * 2 * 512], F8, kind="ExternalInput")
    wfc2_d = nc.dram_tensor("w_fc2", [4, P, KC * 2 * N], F8, kind="ExternalInput")
    bfc1_d = nc.dram_tensor("b_fc1", [HID], F32, kind="ExternalInput")
    ln1s_d = nc.dram_tensor("ln1_scale", [D], F32, kind="ExternalInput")
    ln1b_d = nc.dram_tensor("ln1_bias", [D], F32, kind="ExternalInput")
    ln2s_d = nc.dram_tensor("ln2_scale", [D], F32, kind="ExternalInput")
    ln2b_d = nc.dram_tensor("ln2_bias", [D], F32, kind="ExternalInput")
    bqkc_d = (
        nc.dram_tensor("b_qkc", [P, 16], F32, kind="ExternalInput")
        if not qk_triv else None
    )
    c1_d = nc.dram_tensor("c1", [D], F16, kind="ExternalInput") if apply_c1 else None
    bfc2_d = (
        nc.dram_tensor("b_fc2c", [D], F16, kind="ExternalInput")
        if apply_bfc2 else None
    )
    y_d = nc.dram_tensor("y", [N, D], F16, kind="ExternalOutput")

    wqkv_v = wqkv_d.ap().rearrange("(kc p) n -> p kc n", p=P)
    wproj_v = wproj_d.ap().rearrange("(kc p) n -> p kc n", p=P)

    import contextlib
    with tile.TileContext(nc) as tc, ExitStack() as ctx, \
            contextlib.suppress(_Stop):
        ep = ctx.enter_context
        constp = ep(tc.tile_pool(name="const", bufs=1))
        xload = ep(tc.tile_pool(name="xload", bufs=XLOAD_BUFS))
        x16p = ep(tc.tile_pool(name="x16", bufs=1))
        htmpp = ep(tc.tile_pool(name="htmp", bufs=2))
        hTp = ep(tc.tile_pool(name="hT", bufs=1))
        qTp = ep(tc.tile_pool(name="qT", bufs=1))
        kTp = ep(tc.tile_pool(name="kT", bufs=1))
        vp = ep(tc.tile_pool(name="vv", bufs=1))
        oTp = ep(tc.tile_pool(name="oT", bufs=1))
        probsp = ep(tc.tile_pool(name="probs", bufs=PROBS_BUFS))
        x1p = ep(tc.tile_pool(name="x1", bufs=1))
        h2Tp = ep(tc.tile_pool(name="h2T", bufs=1))
        a1p = ep(tc.tile_pool(name="a1", bufs=1))
        wqp = ep(tc.tile_pool(name="wq", bufs=4))
        w1p = ep(tc.tile_pool(name="w1", bufs=2))
        w2p = ep(tc.tile_pool(name="w2", bufs=2))
        statsp = ep(tc.tile_pool(name="stats", bufs=4))
        otmpp = ep(tc.tile_pool(name="otmp", bufs=2))
        itp = ep(tc.tile_pool(name="it", bufs=2))
        pmp = ep(tc.tile_pool(name="pm", bufs=2, space="PSUM"))
        scp = ep(tc.tile_pool(name="sc", bufs=2, space="PSUM"))
        pvp = ep(tc.tile_pool(name="pv", bufs=2, space="PSUM"))

        # ---- first x tile load before anything ----
        x_t0 = xload.tile([P, D], F32, tag="x_t")
        nc.sync.dma_start(x_t0[:, 0:512], x_d.ap()[ts(0, P), 0:512])
        nc.sync.dma_start(x_t0[:, 512:1024], x_d.ap()[ts(0, P), 512:1024])

        # ---- constants ----
        ident = constp.tile([P, P], F16)
        make_identity(nc, ident[:])
        identsw = constp.tile([P, P], F16)
        make_identity(nc, identsw[:])
        nc.gpsimd.tensor_scalar_mul(identsw[:], identsw[:], SW)
        eps_t = constp.tile([P, 1], F32)
        nc.vector.memset(eps_t[:], EPS)
        ones16 = constp.tile([1, P], F16)
        nc.vector.memset(ones16[:], 1.0)
        ln1s = ln1b = ln2s = ln2b = None
        if not ln1_triv:
            ln1s = constp.tile([P, KC], F32)
            nc.gpsimd.dma_start(ln1s[:], ln1s_d.ap().rearrange("(k p) -> p k", p=P))
            ln1b = constp.tile([P, KC], F32)
            nc.gpsimd.dma_start(ln1b[:], ln1b_d.ap().rearrange("(k p) -> p k", p=P))
        if not ln2_triv:
            ln2s = constp.tile([P, KC], F32)
            nc.gpsimd.dma_start(ln2s[:], ln2s_d.ap().rearrange("(k p) -> p k", p=P))
            ln2b = constp.tile([P, KC], F32)
            nc.gpsimd.dma_start(ln2b[:], ln2b_d.ap().rearrange("(k p) -> p k", p=P))
        bqk = None
        if not qk_triv:
            bqk = constp.tile([P, 16], F32)
            nc.gpsimd.dma_start(bqk[:], bqkc_d.ap())
        bfc1 = constp.tile([P, HID // P], F32)
        nc.gpsimd.dma_start(bfc1[:], bfc1_d.ap().rearrange("(m p) -> p m", p=P))
        if apply_c1:
            c1row = constp.tile([1, D], F16)
            nc.gpsimd.dma_start(c1row[:], c1_d.ap().unsqueeze(0))
        if apply_bfc2:
            b2row = constp.tile([1, D], F16)
            nc.gpsimd.dma_start(b2row[:], bfc2_d.ap().unsqueeze(0))

        x16 = x16p.tile([P, MT, D], F16)
        hT = hTp.tile([P, KC, N], F8, tag="hT")

        def ln_stats(src_ap, mt, out_T, s_cols, b_cols, trivial):
            """LN stats + normalize into htmp; returns h staging tile."""
            st = statsp.tile([P, 2, 6], F32, tag="st")
            xr = src_ap.rearrange("p (a b) -> p a b", b=512)
            nc.vector.bn_stats(st[:, 0, :], xr[:, 0, :])
            nc.vector.bn_stats(st[:, 1, :], xr[:, 1, :])
            mv = statsp.tile([P, 2], F32, tag="mv")
            nc.vector.bn_aggr(mv[:], st[:])
            rstd = statsp.tile([P, 1], F32, tag="rstd")
            nc.scalar.activation(rstd[:], mv[:, 1:2], AF.Sqrt, bias=eps_t[:])
            nc.vector.reciprocal(rstd[:], rstd[:])
            h = htmpp.tile([P, D], F16, tag="h")
            nc.vector.tensor_scalar(
                out=h[:, 0:512], in0=src_ap[:, 0:512], scalar1=mv[:, 0:1],
                scalar2=rstd[:], op0=ALU.subtract, op1=ALU.mult,
            )
            nc.gpsimd.tensor_scalar(
                out=h[:, 512:1024], in0=src_ap[:, 512:1024], scalar1=mv[:, 0:1],
                scalar2=rstd[:], op0=ALU.subtract, op1=ALU.mult,
            )
            return h

        def ln_transp(h, out_T, mt, s_cols, b_cols, trivial, ei, kcs=None):
            """Transpose h chunks into out_T[:, kc, mt-window].

            Trivial path: all transposes land in ONE fp16 psum tile (1 bank)
            followed by a single merged copy."""
            kcs = kcs if kcs is not None else range(KC)
            if trivial:
                tp = scp.tile([P, KC, P], F16, tag="sc", name=f"tp{mt}")
                for kc in kcs:
                    nc.tensor.transpose(tp[:, kc, :], h[:, ts(kc, P)], ident[:])
                eng_copy(nc, ei, out_T[:, :, ts(mt, P)], tp[:])
            else:
                for kc in kcs:
                    pt_t = scp.tile([P, P], F16, tag="sc", name=f"pt{mt}_{kc}")
                    nc.tensor.transpose(pt_t[:], h[:, ts(kc, P)], ident[:])
                    nc.vector.tensor_scalar(
                        out=out_T[:, kc, ts(mt, P)], in0=pt_t[:],
                        scalar1=s_cols[:, kc:kc + 1], scalar2=b_cols[:, kc:kc + 1],
                        op0=ALU.mult, op1=ALU.add,
                    )

        # ---- phase 1: LN1 + transpose + x16 ----
        def ln1_mt(mt):
            if mt == 0:
                x_t = x_t0
            else:
                x_t = xload.tile([P, D], F32, tag="x_t", name=f"x_t{mt}")
                nc.sync.dma_start(x_t[:, 0:512], x_d.ap()[ts(mt, P), 0:512])
                nc.sync.dma_start(x_t[:, 512:1024], x_d.ap()[ts(mt, P), 512:1024])
            eng16 = nc.gpsimd if mt % 2 == 0 else nc.vector
            eng16.tensor_copy(x16[:, mt, :], x_t[:])
            h = ln_stats(x_t[:], mt, hT, ln1s, ln1b, ln1_triv)
            ln_transp(h, hT, mt, ln1s, ln1b, ln1_triv, mt % 2)

        for mt in range(4):
            ln1_mt(mt)

        # ---- phase 2: qkv (nt=0 overlaps LN1 of tiles 4..7) ----
        qT = qTp.tile([P, 4, 2, N], F8, tag="qT")
        kT = kTp.tile([P, 4, 2, N], F8, tag="kT")
        v_sb = vp.tile([P, MT, HEADS * VP], F8, tag="vv")

        def wpiece(view, n0, w=512):
            t = wqp.tile([P, KC, 512], F8, tag="w")
            nc.sync.dma_start(t[:, :, 0:w], view[:, :, n0:n0 + w])
            return t

        def qkv_group_nt(piece, col_l, dst, gi, nt):
            """One (T, s) q/k group for one token half -> dst fp8."""
            pm = pmp.tile([P, 512], F32, tag="pm", name=f"qk{gi}_{nt}")
            for kk in range(4):
                nc.tensor.matmul(
                    pm[:],
                    piece[:, 2 * kk:2 * kk + 2, ts(col_l, P)],
                    hT[:, 2 * kk:2 * kk + 2, ts(nt, 512)],
                    start=(kk == 0), stop=(kk == 3), perf_mode=DR,
                )
            if qk_triv:
                scaled_copy(nc, gi + nt, dst[:, ts(nt, 512)], pm[:], RSW)
            else:
                nc.vector.tensor_scalar(
                    out=dst[:, ts(nt, 512)], in0=pm[:],
                    scalar1=bqk[:, gi:gi + 1], scalar2=RSW,
                    op0=ALU.add, op1=ALU.mult,
                )

        qk_pieces = []
        for half in (1, 0):  # k first so scores can start early
            for piece_i in range(2):
                qk_pieces.append(wpiece(wqkv_v, half * 1024 + piece_i * 512))

        ln1_left = list(range(4, 8))
        gi = 0
        for nt in range(NT):
            for pi, piece in enumerate(qk_pieces):
                for lT in range(2):
                    for s in range(2):
                        half = 1 - pi // 2
                        T = (pi % 2) * 2 + lT
                        dst_t = kT if half == 1 else qT
                        qkv_group_nt(piece, lT * 2 + s, dst_t[:, T, s, :],
                                     (1 - half) * 8 + T * 2 + s, nt)
                if nt == 0 and ln1_left:
                    ln1_mt(ln1_left.pop(0))

        mark("ln1")
        mark("qkv_qk")
        v_pieces = [wpiece(wqkv_v, n0) for n0 in (2048, 2560)]

        def v_group(mt):
            v_row = v_sb[:, mt, :].rearrange("p (h c) -> p h c", c=VP)
            nc.vector.memset(v_row[:, :, HD:HD + 1], 1.0)
            pm = pmp.tile([P, N], F32, tag="pm")
            for vh in range(2):
                for kk in range(4):
                    nc.tensor.matmul(
                        pm[:, ts(vh, 512)],
                        hT[:, 2 * kk:2 * kk + 2, ts(mt, P)],
                        v_pieces[vh][:, 2 * kk:2 * kk + 2, :],
                        start=(kk == 0), stop=(kk == 3), perf_mode=DR,
                    )
            src = pm[:].rearrange("p (h c) -> p h c", c=HD)
            scaled_copy(nc, mt, v_row[:, :, 0:HD], src, RSW)

        # ---- attention machinery ----
        def probs_alloc(i, qw):
            if i % 3 == 2:
                return hTp.tile([P, KC, qw], F8, tag="hT", name=f"pb{i}")
            return probsp.tile([P, KC, qw], F8, tag="probs", name=f"pb{i}")

        def emit_exp(engc, dst, src_ps):
            if engc == "A":
                nc.scalar.activation(dst, src_ps, AF.Exp, scale=SCALE)
            else:
                it = itp.tile([P, src_ps.shape[-1]], I32, tag="it", name="it")
                nc.vector.tensor_scalar(
                    out=it[:], in0=src_ps, scalar1=EA, scalar2=EB,
                    op0=ALU.mult, op1=ALU.add,
                )
                nc.gpsimd.tensor_copy(dst, it[:].bitcast(F32))

        def scores_head(h, qoff, qw, probs_h, pat, use_pm, pull=None):
            T, g = h // 4, h % 4
            pm_t = [None]
            for mk in range(MT):
                if use_pm and mk % 4 == 2:
                    pm_t[0] = pmp.tile([P, N], F32, tag="pm", name=f"spm{h}_{mk}")
                if use_pm and mk % 4 >= 2:
                    sp = pm_t[0][:, ts(mk % 2, 512)][:, 0:qw]
                else:
                    spt = scp.tile([P, qw], F32, tag="sc", name=f"sp{h}_{mk}")
                    sp = spt[:]
                nc.tensor.matmul(
                    sp, kT[32 * g:32 * g + 32, T, :, ts(mk, P)],
                    qT[32 * g:32 * g + 32, T, :, qoff:qoff + qw],
                    start=True, stop=True, perf_mode=DR,
                    tile_position=(32 * g, 0),
                )
                emit_exp(pat[mk], probs_h[:, mk, :], sp)
                if pull is not None and mk % PULL_MOD == PULL_MOD - 1:
                    pull(1)

        def pav_head(h, qoff, qw, probs_h):
            mc_h = h // 2
            pv = pvp.tile([P, qw], F32, tag="pv", name="pv")
            for j in range(4):
                nc.tensor.matmul(
                    pv[0:HD + 1, :],
                    v_sb[:, 2 * j:2 * j + 2, h * VP:h * VP + HD + 1],
                    probs_h[:, 2 * j:2 * j + 2, :],
                    start=(j == 0), stop=(j == 3), perf_mode=DR,
                    skip_group_check=True,
                )
            srow = otmpp.tile([1, qw], F16, tag="srow", name="srow")
            with nc.allow_low_precision(reason="softmax denom ~1e3, fp16 ok"):
                nc.vector.reciprocal(srow[:], pv[HD:HD + 1, :])
            # broadcast 1/denom across 64 partitions (gpsimd custom op)
            rb = otmpp.tile([HD, qw], F16, tag="rb", name="rb")
            nc.gpsimd.partition_broadcast(rb[:], srow[:], channels=HD)
            if h % 2 == 0:
                nc.vector.tensor_mul(
                    oT[0:HD, mc_h, qoff:qoff + qw], pv[0:HD, :], rb[:]
                )
            else:
                o_t = otmpp.tile([HD, qw], F8, tag="o_t", name="o_t")
                nc.vector.tensor_mul(o_t[:], pv[0:HD, :], rb[:])
                nc.gpsimd.dma_start(oT[HD:P, mc_h, qoff:qoff + qw], o_t[:])

        oT = oTp.tile([P, KC, N], F8, tag="oT")
        x1 = x1p.tile([P, MT, D], F16)
        h2T = h2Tp.tile([P, KC, N], F8, tag="h2T")
        a1 = a1p.tile([P, 4, KC, N], F8)

        # ---- mlp generators (thunk steps pulled between score matmuls) ----
        proj_pieces = [None, None]

        def gen_proj(mt):
            pm = pmp.tile([P, N], F32, tag="pm", name=f"projpm{mt}")
            for kk in range(4):
                nc.tensor.matmul(
                    pm[:, 0:512], oT[:, 2 * kk:2 * kk + 2, ts(mt, P)],
                    proj_pieces[0][:, 2 * kk:2 * kk + 2, :],
                    start=(kk == 0), stop=(kk == 3), perf_mode=DR,
                )
            yield
            for kk in range(4):
                nc.tensor.matmul(
                    pm[:, 512:1024], oT[:, 2 * kk:2 * kk + 2, ts(mt, P)],
                    proj_pieces[1][:, 2 * kk:2 * kk + 2, :],
                    start=(kk == 0), stop=(kk == 3), perf_mode=DR,
                )
            for ph in range(2):
                nc.tensor.matmul(pm[:, ts(ph, 512)], identsw[:],
                                 x16[:, mt, ts(ph, 512)], start=False,
                                 stop=True, skip_group_check=True)
                if apply_c1:
                    nc.tensor.matmul(pm[:, ts(ph, 512)], ones16[0:1, :],
                                     c1row[:, ts(ph, 512)], start=False,
                                     stop=True, skip_group_check=True)
            yield
            eng_copy(nc, mt, x1[:, mt, :], pm[:])
            yield
            h = ln_stats(x1[:, mt, :], mt, h2T, ln2s, ln2b, ln2_triv)
            yield
            ln_transp(h, h2T, mt, ln2s, ln2b, ln2_triv, mt % 2)

        def gen_fc1(p8, qoff, qw):
            w1_t = w1p.tile([P, KC, 2, 512], F8, tag="w1", name=f"w1p{p8}")
            nc.sync.dma_start(
                w1_t[:].rearrange("p a b c -> p (a b c)"), wfc1_d.ap()[p8]
            )
            bp = min(4, N // qw)     # mh blocks packed per pm tile
            gelu_todo = None

            def emit_gelu(pm, pmi):
                mhg0 = p8 * 4 + pmi * bp
                if fc1b_triv:
                    nc.scalar.activation(
                        a1[:, mhg0 // 8, mhg0 % 8:mhg0 % 8 + bp, qoff:qoff + qw],
                        pm[:, 0:bp * qw].rearrange("p (a b) -> p a b", b=qw),
                        AF.Gelu_apprx_tanh, bias=0.0, scale=RSW,
                    )
                else:
                    for mi in range(bp):
                        mhg = mhg0 + mi
                        nc.scalar.activation(
                            a1[:, mhg // 8, mhg % 8, qoff:qoff + qw],
                            pm[:, mi * qw:(mi + 1) * qw],
                            AF.Gelu_apprx_tanh, bias=bfc1[:, mhg:mhg + 1],
                            scale=RSW,
                        )

            for pmi in range(4 // bp):
                pm = pmp.tile([P, N], F32, tag="pm", name=f"fc1pm{p8}_{pmi}")
                for mi in range(bp):
                    mh_l = pmi * bp + mi
                    for kc in range(KC):
                        nc.tensor.matmul(
                            pm[:, mi * qw:(mi + 1) * qw],
                            w1_t[:, kc, :, ts(mh_l, P)],
                            slot0(h2T[:, kc, qoff:qoff + qw], qw),
                            start=(kc == 0), stop=(kc == KC - 1), perf_mode=DR,
                        )
                yield
                emit_gelu(pm, pmi)

        def w2_piece(g, i):
            t = w2p.tile([P, KC, 2, N], F8, tag="w2", name=f"w2g{g}_{i}")
            nc.sync.dma_start(
                t[:].rearrange("p a b c -> p (a b c)"), wfc2_d.ap()[g]
            )
            return t

        def gen_fc2(mt, gpair, w2ab, final):
            pm = pmp.tile([P, N], F32, tag="pm", name=f"fc2pm{mt}_{gpair}")
            for ph in range(2):
                for gl in range(2):
                    g = gpair * 2 + gl
                    for kc in range(KC):
                        nc.tensor.matmul(
                            pm[:, ts(ph, 512)],
                            slot0(a1[:, g, kc, ts(mt, P)], P),
                            w2ab[gl][:, kc, :, ts(ph, 512)],
                            start=(gl == 0 and kc == 0),
                            stop=(gl == 1 and kc == KC - 1), perf_mode=DR,
                        )
                # x1 already holds SW*x1 (after pass 1, SW*(x1+mlp_g01))
                nc.tensor.matmul(pm[:, ts(ph, 512)], ident[:],
                                 x1[:, mt, ts(ph, 512)], start=False,
                                 stop=True, skip_group_check=True)
                if final and apply_bfc2:
                    nc.tensor.matmul(pm[:, ts(ph, 512)], ones16[0:1, :],
                                     b2row[:, ts(ph, 512)], start=False,
                                     stop=True, skip_group_check=True)
                if ph == 0:
                    yield
            yield
            if not final:
                eng_copy(nc, mt, x1[:, mt, :], pm[:])
            else:
                y16 = xload.tile([P, D], F16, tag="x_t", name="y16")
                scaled_copy(nc, mt, y16[:], pm[:], RSW)
                nc.sync.dma_start(y_d.ap()[ts(mt, P), :], y16[:])

        # ======= phases 3-5: attention chunks 512/256/256 + pipelined mlp ===
        from collections import deque
        probs_n = [0]

        def next_probs(qw):
            t = probs_alloc(probs_n[0], qw)
            probs_n[0] += 1
            return t

        # ---- phase 3: attention chunk A (queries 0:512) ----
        probs_q = []
        probs_q.append(next_probs(512))
        scores_head(0, 0, 512, probs_q[0], EXP_PAT_A, False)
        for mt in range(4):
            v_group(mt)
        probs_q.append(next_probs(512))
        scores_head(1, 0, 512, probs_q[1], EXP_PAT_A, False)
        for mt in range(4, 8):
            v_group(mt)
        for h in range(2, HEADS):
            probs_q.append(next_probs(512))
            scores_head(h, 0, 512, probs_q[-1], EXP_PAT_A, True)
            pav_head(h - 2, 0, 512, probs_q.pop(0))
        pav_head(HEADS - 2, 0, 512, probs_q.pop(0))
        pav_head(HEADS - 1, 0, 512, probs_q.pop(0))

        mark("attnA")
        # proj + fc2(g0,g1) weights land during attention A
        proj_pieces[0] = wpiece(wproj_v, 0)
        proj_pieces[1] = wpiece(wproj_v, 512)
        w2_01 = [w2_piece(0, 0), w2_piece(1, 0)]

        work = deque()

        def pull(k):
            for _ in range(k):
                while work:
                    try:
                        next(work[0])
                        break
                    except StopIteration:
                        work.popleft()

        def attn_chunk(qoff, qw, pat, late=None):
            probs_q = []
            for h in range(HEADS):
                probs_q.append(next_probs(qw))
                scores_head(h, qoff, qw, probs_q[-1], pat, h >= USE_PM_B, pull)
                if h >= 2:
                    pav_head(h - 2, qoff, qw, probs_q.pop(0))
                if late is not None and h == late[0]:
                    late[1]()
                pull(1)
            pav_head(HEADS - 2, qoff, qw, probs_q.pop(0))
            pav_head(HEADS - 1, qoff, qw, probs_q.pop(0))

        # ---- phase 4: attention B (512:1024) || mlp tokens 0:512 ----
        w2_23 = []
        for mt in range(4):
            work.append(gen_proj(mt))
        for p8 in range(8):
            work.append(gen_fc1(p8, 0, 512))
        for mt in range(4):
            work.append(gen_fc2(mt, 0, w2_01, final=False))

        def add_fc2b_t1():
            w2_23.extend([w2_piece(2, 0), w2_piece(3, 0)])
            for mt in range(4):
                work.append(gen_fc2(mt, 1, w2_23, final=True))

        attn_chunk(512, 512, EXP_PAT_B, late=(8, add_fc2b_t1))
        # prefetch phase-5 weights while attention B drains
        w2_01b = [w2_piece(0, 1), w2_piece(1, 1)]
        while work:
            pull(1)

        mark("phase4")
        # ---- phase 5: mlp tokens 512:1024 ----
        for mt in range(4, 6):
            for _ in gen_proj(mt):
                pass
        w2_23b = [w2_piece(2, 1), w2_piece(3, 1)]
        for mt in range(6, 8):
            for _ in gen_proj(mt):
                pass
        for p8 in range(8):
            for _ in gen_fc1(p8, 512, 512):
                pass
        for mt in range(4, 8):
            for _ in gen_fc2(mt, 0, w2_01b, final=False):
                pass
        for mt in range(4, 8):
            for _ in gen_fc2(mt, 1, w2_23b, final=True):
                pass
        mark("end")

    nc.compile()
    return nc


_cache = {}
_last_inmaps = None


def _get_nc(*key):
    if key not in _cache:
        _cache[key] = build_block(*key)
    return _cache[key]


def _f8(a):
    import ml_dtypes
    return np.clip(np.asarray(a, np.float32), -240.0, 240.0).astype(
        ml_dtypes.float8_e4m3
    )


def _perm_qk():
    perm = np.empty(D, np.int64)
    i = 0
    for T in range(4):
        for s in range(2):
            for g in range(4):
                for p in range(32):
                    perm[i] = (4 * T + g) * HD + 32 * s + p
                    i += 1
    return perm


def kernel(
    x, w_qkv, b_qkv, w_proj, b_proj, ln1_scale, ln1_bias,
    ln2_scale, ln2_bias, w_fc1, b_fc1, w_fc2, b_fc2,
):
    x = np.asarray(x, np.float32)
    B = x.shape[0]
    b_qkv = np.asarray(b_qkv, np.float32)
    b_v = b_qkv[2 * D:]
    c1 = b_v.astype(np.float64) @ np.asarray(w_proj, np.float64) + np.asarray(
        b_proj, np.float64
    )
    c1 = c1.astype(np.float32)
    bfc2 = np.asarray(b_fc2, np.float32)
    ln1_scale = np.asarray(ln1_scale, np.float32)
    ln1_bias = np.asarray(ln1_bias, np.float32)
    ln2_scale = np.asarray(ln2_scale, np.float32)
    ln2_bias = np.asarray(ln2_bias, np.float32)
    ln1_triv = bool(np.all(ln1_scale == 1) and np.all(ln1_bias == 0))
    ln2_triv = bool(np.all(ln2_scale == 1) and np.all(ln2_bias == 0))
    qk_triv = bool(np.all(b_qkv[:2 * D] == 0))
    apply_c1 = bool(np.any(c1 != 0))
    apply_bfc2 = bool(np.any(bfc2 != 0))
    fc1b_triv = bool(np.all(np.asarray(b_fc1, np.float32) == 0))

    nc = _get_nc(ln1_triv, ln2_triv, qk_triv, apply_c1, apply_bfc2, fc1b_triv)

    perm = _perm_qk()
    w_qkv = np.asarray(w_qkv, np.float32)
    wq = w_qkv[:, :D][:, perm]
    wk = w_qkv[:, D:2 * D][:, perm]
    wv = w_qkv[:, 2 * D:]
    wqkv_p = np.concatenate([wq, wk, wv], axis=1)

    w_fc1 = np.asarray(w_fc1, np.float32) * SW
    w1_hi = _f8(w_fc1)
    w1_lo = _f8(w_fc1 - w1_hi.astype(np.float32))
    # [D, 2, HID] -> [piece, p, (kc two col)]
    w1s = np.stack([w1_hi, w1_lo], axis=1)
    w1s = w1s.reshape(KC, P, 2, 8, 512).transpose(3, 1, 0, 2, 4)
    w1s = np.ascontiguousarray(w1s.reshape(8, P, KC * 2 * 512))
    w_fc2 = np.asarray(w_fc2, np.float32) * SW
    w2_hi = _f8(w_fc2)
    w2_lo = _f8(w_fc2 - w2_hi.astype(np.float32))
    # [HID, 2, D] -> [g, p, (kc two col)]
    w2s = np.stack([w2_hi, w2_lo], axis=1)
    w2s = w2s.reshape(4, KC, P, 2, N).transpose(0, 2, 1, 3, 4)
    w2s = np.ascontiguousarray(w2s.reshape(4, P, KC * 2 * N))

    base = {
        "w_qkv": _f8(wqkv_p * SW),
        "w_proj": _f8(np.asarray(w_proj, np.float32) * SW),
        "w_fc1": w1s,
        "w_fc2": w2s,
        "b_fc1": np.asarray(b_fc1, np.float32),
        "ln1_scale": ln1_scale,
        "ln1_bias": ln1_bias,
        "ln2_scale": ln2_scale,
        "ln2_bias": ln2_bias,
    }
    if not qk_triv:
        bqk = b_qkv[:2 * D][np.concatenate([perm, D + perm])] * SW
        base["b_qkc"] = np.ascontiguousarray(
            bqk.reshape(16, P).T.astype(np.float32)
        )
    if apply_c1:
        base["c1"] = (c1 * SW).astype(np.float16)
    if apply_bfc2:
        base["b_fc2c"] = (bfc2 * SW).astype(np.float16)

    in_maps = [dict(base, x=np.ascontiguousarray(x[i])) for i in range(B)]
    global _last_inmaps
    _last_inmaps = in_maps
    last_err = None
    for _attempt in range(3):
        try:
            res = run_bass_kernel_spmd(nc, in_maps, core_ids=list(range(B)))
            break
        except Exception as e:  # transient NRT/axon worker failures
            last_err = e
            import time as _time

            _time.sleep(2.0)
    else:
        raise last_err
    out = np.stack([res.results[i]["y"] for i in range(B)], axis=0)
    return np.ascontiguousarray(out.astype(np.float32))


# revision 3
# speedup vs baseline: 1.0134x; 1.0134x over previous
"""Trainium2 Bass kernel v2 for the dense transformer block (pre-LN MHA + MLP).

Data parallel across 8 cores (batch -> core). Per core [1024, 1024]:

- fp8e4 DoubleRow matmuls everywhere the contraction allows:
  qkv / proj (k-chunk pairs), scores (head-dim split 2x32 across slot dim,
  32-partition stationary), pav (key-chunk pairs), fc1/fc2 (weight hi+lo
  fp8 split in the slot dim, activation streamed to both slots with a
  0-stride AP).
- Weights scaled x1024 on host (fp8 subnormal escape); descale folded into
  epilogues; x1 kept scaled by 1024 (LN is scale-invariant) so the residual
  adds are PE identity-matmuls and the only descale is the final y copy.
- Softmax exp split across ACT (hw exp) and DVE/GPSIMD (Schraudolph bit
  trick, fine since softmax normalizes away small relative error).
- Attention + MLP processed in query halves so the second half's exp wall
  overlaps the first half's MLP matmuls.
"""

from contextlib import ExitStack

import numpy as np

import concourse.bass as bass
import concourse.tile as tile
from concourse import bacc, mybir
from concourse.bass import ts
from concourse.bass_utils import run_bass_kernel_spmd
from concourse.masks import make_identity

F32 = mybir.dt.float32
F16 = mybir.dt.float16
F8 = mybir.dt.float8e4
I32 = mybir.dt.int32
AF = mybir.ActivationFunctionType
ALU = mybir.AluOpType
DR = mybir.MatmulPerfMode.DoubleRow

P = 128
N = 1024         # tokens per core
D = 1024
KC = 8           # contraction chunks of 128 over D
HEADS = 16
HD = 64
HID = 4096
EPS = 1e-6
NT = 2
MT = 8
QH = 512         # query half
SCALE = HD ** -0.5
SW = 1024.0      # weight scale (fp8)
RSW = 1.0 / SW
VP = 65          # v_sb per-head pitch: 64 dims + ones col (mk step 16*65)

# Schraudolph exp: exp(SCALE*x) ~ bitcast_f32(int32(EA*x + EB))
EA = (2 ** 23) / np.log(2.0) * SCALE
EB = 127.0 * (2 ** 23) - 486411.0

# exp engine split per (mk index 0..7): A=ACT hw exp, D=DVE+GPSIMD split
EXP_PAT_A = ["A", "D", "A", "A", "A", "D", "A", "A"]  # phase A (no mlp rival)
EXP_PAT_B = ["A", "D", "A", "A", "D", "A", "A", "D"]  # phase B (gelu on ACT)
PULL_MOD = 4       # pull a thunk after every PULL_MOD-th mk in scores_head
USE_PM_B = 99      # attnB heads >= this borrow pm psum slots for scores
PROBS_BUFS = 2
XLOAD_BUFS = 2


def slot0(ap_, n):
    """[p, 2(0-stride), n] view of a [p, n] AP: stream same data to both
    DoubleRow slots."""
    return bass.AP(
        tensor=ap_.tensor, offset=ap_.offset,
        ap=[list(ap_.ap[0]), [0, 2], [ap_.ap[-1][0], n]],
    )


def scaled_copy(nc, ei, dst, src, s):
    """dst = src * s on DVE (even) or ACT (odd). GPSIMD cannot read PSUM."""
    if ei % 2 == 0:
        nc.vector.tensor_scalar_mul(dst, src, s)
    else:
        nc.scalar.activation(dst, src, AF.Copy, scale=s)


def eng_copy(nc, ei, dst, src):
    if ei % 2 == 0:
        nc.vector.tensor_copy(dst, src)
    else:
        nc.scalar.copy(dst, src)


def build_block(ln1_triv, ln2_triv, qk_triv, apply_c1, apply_bfc2,
                fc1b_triv=True, stop_after=None, phase_marks=None):
    nc = bacc.Bacc("TRN2", target_bir_lowering=False, debug=False, num_devices=8)

    class _Stop(Exception):
        pass

    def mark(name):
        if phase_marks is not None:
            phase_marks.append((name, len(nc.m.functions[0].blocks[0].instructions)))
        if stop_after is not None and name == stop_after:
            raise _Stop()

    x_d = nc.dram_tensor("x", [N, D], F32, kind="ExternalInput")
    wqkv_d = nc.dram_tensor("w_qkv", [D, 3 * D], F8, kind="ExternalInput")
    wproj_d = nc.dram_tensor("w_proj", [D, D], F8, kind="ExternalInput")
    # piece-contiguous: [piece, p, (kc two col)] so each stream is a 2D DMA
    wfc1_d = nc.dram_tensor("w_fc1", [8, P, KC * 2 * 512], F8, kind="ExternalInput")
    wfc2_d = nc.dram_tensor("w_fc2", [4, P, KC * 2 * N], F8, kind="ExternalInput")
    bfc1_d = nc.dram_tensor("b_fc1", [HID], F32, kind="ExternalInput")
    ln1s_d = nc.dram_tensor("ln1_scale", [D], F32, kind="ExternalInput")
    ln1b_d = nc.dram_tensor("ln1_bias", [D], F32, kind="ExternalInput")
    ln2s_d = nc.dram_tensor("ln2_scale", [D], F32, kind="ExternalInput")
    ln2b_d = nc.dram_tensor("ln2_bias", [D], F32, kind="ExternalInput")
    bqkc_d = (
        nc.dram_tensor("b_qkc", [P, 16], F32, kind="ExternalInput")
        if not qk_triv else None
    )
    c1_d = nc.dram_tensor("c1", [D], F16, kind="ExternalInput") if apply_c1 else None
    bfc2_d = (
        nc.dram_tensor("b_fc2c", [D], F16, kind="ExternalInput")
        if apply_bfc2 else None
    )
    y_d = nc.dram_tensor("y", [N, D], F16, kind="ExternalOutput")

    wqkv_v = wqkv_d.ap().rearrange("(kc p) n -> p kc n", p=P)
    wproj_v = wproj_d.ap().rearrange("(kc p) n -> p kc n", p=P)

    import contextlib
    with tile.TileContext(nc) as tc, ExitStack() as ctx, \
            contextlib.suppress(_Stop):
        ep = ctx.enter_context
        constp = ep(tc.tile_pool(name="const", bufs=1))
        xload = ep(tc.tile_pool(name="xload", bufs=XLOAD_BUFS))
        x16p = ep(tc.tile_pool(name="x16", bufs=1))
        htmpp = ep(tc.tile_pool(name="htmp", bufs=2))
        hTp = ep(tc.tile_pool(name="hT", bufs=1))
        qTp = ep(tc.tile_pool(name="qT", bufs=1))
        kTp = ep(tc.tile_pool(name="kT", bufs=1))
        vp = ep(tc.tile_pool(name="vv", bufs=1))
        oTp = ep(tc.tile_pool(name="oT", bufs=1))
        probsp = ep(tc.tile_pool(name="probs", bufs=PROBS_BUFS))
        x1p = ep(tc.tile_pool(name="x1", bufs=1))
        h2Tp = ep(tc.tile_pool(name="h2T", bufs=1))
        a1p = ep(tc.tile_pool(name="a1", bufs=1))
        wqp = ep(tc.tile_pool(name="wq", bufs=4))
        w1p = ep(tc.tile_pool(name="w1", bufs=2))
        w2p = ep(tc.tile_pool(name="w2", bufs=2))
        statsp = ep(tc.tile_pool(name="stats", bufs=4))
        otmpp = ep(tc.tile_pool(name="otmp", bufs=2))
        itp = ep(tc.tile_pool(name="it", bufs=2))
        pmp = ep(tc.tile_pool(name="pm", bufs=2, space="PSUM"))
        scp = ep(tc.tile_pool(name="sc", bufs=3, space="PSUM"))
        pvp = ep(tc.tile_pool(name="pv", bufs=1, space="PSUM"))

        # ---- first x tile load before anything ----
        x_t0 = xload.tile([P, D], F32, tag="x_t")
        nc.sync.dma_start(x_t0[:, 0:512], x_d.ap()[ts(0, P), 0:512])
        nc.sync.dma_start(x_t0[:, 512:1024], x_d.ap()[ts(0, P), 512:1024])

        # ---- constants ----
        ident = constp.tile([P, P], F16)
        make_identity(nc, ident[:])
        identsw = constp.tile([P, P], F16)
        make_identity(nc, identsw[:])
        nc.gpsimd.tensor_scalar_mul(identsw[:], identsw[:], SW)
        eps_t = constp.tile([P, 1], F32)
        nc.vector.memset(eps_t[:], EPS)
        ones16 = constp.tile([1, P], F16)
        nc.vector.memset(ones16[:], 1.0)
        ln1s = ln1b = ln2s = ln2b = None
        if not ln1_triv:
            ln1s = constp.tile([P, KC], F32)
            nc.gpsimd.dma_start(ln1s[:], ln1s_d.ap().rearrange("(k p) -> p k", p=P))
            ln1b = constp.tile([P, KC], F32)
            nc.gpsimd.dma_start(ln1b[:], ln1b_d.ap().rearrange("(k p) -> p k", p=P))
        if not ln2_triv:
            ln2s = constp.tile([P, KC], F32)
            nc.gpsimd.dma_start(ln2s[:], ln2s_d.ap().rearrange("(k p) -> p k", p=P))
            ln2b = constp.tile([P, KC], F32)
            nc.gpsimd.dma_start(ln2b[:], ln2b_d.ap().rearrange("(k p) -> p k", p=P))
        bqk = None
        if not qk_triv:
            bqk = constp.tile([P, 16], F32)
            nc.gpsimd.dma_start(bqk[:], bqkc_d.ap())
        bfc1 = constp.tile([P, HID // P], F32)
        nc.gpsimd.dma_start(bfc1[:], bfc1_d.ap().rearrange("(m p) -> p m", p=P))
        if apply_c1:
            c1row = constp.tile([1, D], F16)
            nc.gpsimd.dma_start(c1row[:], c1_d.ap().unsqueeze(0))
        if apply_bfc2:
            b2row = constp.tile([1, D], F16)
            nc.gpsimd.dma_start(b2row[:], bfc2_d.ap().unsqueeze(0))

        x16 = x16p.tile([P, MT, D], F16)
        hT = hTp.tile([P, KC, N], F8, tag="hT")

        def ln_stats(src_ap, mt, out_T, s_cols, b_cols, trivial):
            """LN stats + normalize into htmp; returns h staging tile."""
            st = statsp.tile([P, 2, 6], F32, tag="st")
            xr = src_ap.rearrange("p (a b) -> p a b", b=512)
            nc.vector.bn_stats(st[:, 0, :], xr[:, 0, :])
            nc.vector.bn_stats(st[:, 1, :], xr[:, 1, :])
            mv = statsp.tile([P, 2], F32, tag="mv")
            nc.vector.bn_aggr(mv[:], st[:])
            rstd = statsp.tile([P, 1], F32, tag="rstd")
            nc.scalar.activation(rstd[:], mv[:, 1:2], AF.Sqrt, bias=eps_t[:])
            nc.vector.reciprocal(rstd[:], rstd[:])
            h = htmpp.tile([P, D], F16, tag="h")
            nc.vector.tensor_scalar(
                out=h[:, 0:512], in0=src_ap[:, 0:512], scalar1=mv[:, 0:1],
                scalar2=rstd[:], op0=ALU.subtract, op1=ALU.mult,
            )
            nc.gpsimd.tensor_scalar(
                out=h[:, 512:1024], in0=src_ap[:, 512:1024], scalar1=mv[:, 0:1],
                scalar2=rstd[:], op0=ALU.subtract, op1=ALU.mult,
            )
            return h

        def ln_transp(h, out_T, mt, s_cols, b_cols, trivial, ei, kcs=None):
            """Transpose h chunks into out_T[:, kc, mt-window].

            Trivial path: all transposes land in ONE fp16 psum tile (1 bank)
            followed by a single merged copy."""
            kcs = kcs if kcs is not None else range(KC)
            if trivial:
                tp = scp.tile([P, KC, P], F16, tag="sc", name=f"tp{mt}")
                for kc in kcs:
                    nc.tensor.transpose(tp[:, kc, :], h[:, ts(kc, P)], ident[:])
                eng_copy(nc, ei, out_T[:, :, ts(mt, P)], tp[:])
            else:
                for kc in kcs:
                    pt_t = scp.tile([P, P], F16, tag="sc", name=f"pt{mt}_{kc}")
                    nc.tensor.transpose(pt_t[:], h[:, ts(kc, P)], ident[:])
                    nc.vector.tensor_scalar(
                        out=out_T[:, kc, ts(mt, P)], in0=pt_t[:],
                        scalar1=s_cols[:, kc:kc + 1], scalar2=b_cols[:, kc:kc + 1],
                        op0=ALU.mult, op1=ALU.add,
                    )

        # ---- phase 1: LN1 + transpose + x16 ----
        def ln1_mt(mt):
            if mt == 0:
                x_t = x_t0
            else:
                x_t = xload.tile([P, D], F32, tag="x_t", name=f"x_t{mt}")
                nc.sync.dma_start(x_t[:, 0:512], x_d.ap()[ts(mt, P), 0:512])
                nc.sync.dma_start(x_t[:, 512:1024], x_d.ap()[ts(mt, P), 512:1024])
            eng16 = nc.gpsimd if mt % 2 == 0 else nc.vector
            eng16.tensor_copy(x16[:, mt, :], x_t[:])
            h = ln_stats(x_t[:], mt, hT, ln1s, ln1b, ln1_triv)
            ln_transp(h, hT, mt, ln1s, ln1b, ln1_triv, mt % 2)

        for mt in range(4):
            ln1_mt(mt)

        # ---- phase 2: qkv (nt=0 overlaps LN1 of tiles 4..7) ----
        qT = qTp.tile([P, 4, 2, N], F8, tag="qT")
        kT = kTp.tile([P, 4, 2, N], F8, tag="kT")
        v_sb = vp.tile([P, MT, HEADS * VP], F8, tag="vv")

        def wpiece(view, n0, w=512):
            t = wqp.tile([P, KC, 512], F8, tag="w")
            nc.sync.dma_start(t[:, :, 0:w], view[:, :, n0:n0 + w])
            return t

        def qkv_group_nt(piece, col_l, dst, gi, nt):
            """One (T, s) q/k group for one token half -> dst fp8."""
            pm = pmp.tile([P, 512], F32, tag="pm", name=f"qk{gi}_{nt}")
            for kk in range(4):
                nc.tensor.matmul(
                    pm[:],
                    piece[:, 2 * kk:2 * kk + 2, ts(col_l, P)],
                    hT[:, 2 * kk:2 * kk + 2, ts(nt, 512)],
                    start=(kk == 0), stop=(kk == 3), perf_mode=DR,
                )
            if qk_triv:
                scaled_copy(nc, gi + nt, dst[:, ts(nt, 512)], pm[:], RSW)
            else:
                nc.vector.tensor_scalar(
                    out=dst[:, ts(nt, 512)], in0=pm[:],
                    scalar1=bqk[:, gi:gi + 1], scalar2=RSW,
                    op0=ALU.add, op1=ALU.mult,
                )

        qk_pieces = []
        for half in (1, 0):  # k first so scores can start early
            for piece_i in range(2):
                qk_pieces.append(wpiece(wqkv_v, half * 1024 + piece_i * 512))

        ln1_left = list(range(4, 8))
        gi = 0
        for nt in range(NT):
            for pi, piece in enumerate(qk_pieces):
                for lT in range(2):
                    for s in range(2):
                        half = 1 - pi // 2
                        T = (pi % 2) * 2 + lT
                        dst_t = kT if half == 1 else qT
                        qkv_group_nt(piece, lT * 2 + s, dst_t[:, T, s, :],
                                     (1 - half) * 8 + T * 2 + s, nt)
                if nt == 0 and ln1_left:
                    ln1_mt(ln1_left.pop(0))

        mark("ln1")
        mark("qkv_qk")
        v_pieces = [wpiece(wqkv_v, n0) for n0 in (2048, 2560)]

        def v_group(mt):
            v_row = v_sb[:, mt, :].rearrange("p (h c) -> p h c", c=VP)
            nc.vector.memset(v_row[:, :, HD:HD + 1], 1.0)
            pm = pmp.tile([P, N], F32, tag="pm")
            for vh in range(2):
                for kk in range(4):
                    nc.tensor.matmul(
                        pm[:, ts(vh, 512)],
                        hT[:, 2 * kk:2 * kk + 2, ts(mt, P)],
                        v_pieces[vh][:, 2 * kk:2 * kk + 2, :],
                        start=(kk == 0), stop=(kk == 3), perf_mode=DR,
                    )
            src = pm[:].rearrange("p (h c) -> p h c", c=HD)
            scaled_copy(nc, mt, v_row[:, :, 0:HD], src, RSW)

        # ---- attention machinery ----
        def probs_alloc(i, qw):
            if i % 3 == 2:
                return hTp.tile([P, KC, qw], F8, tag="hT", name=f"pb{i}")
            return probsp.tile([P, KC, qw], F8, tag="probs", name=f"pb{i}")

        def emit_exp(engc, dst, src_ps):
            if engc == "A":
                nc.scalar.activation(dst, src_ps, AF.Exp, scale=SCALE)
            else:
                it = itp.tile([P, src_ps.shape[-1]], I32, tag="it", name="it")
                nc.vector.tensor_scalar(
                    out=it[:], in0=src_ps, scalar1=EA, scalar2=EB,
                    op0=ALU.mult, op1=ALU.add,
                )
                nc.gpsimd.tensor_copy(dst, it[:].bitcast(F32))

        def scores_head(h, qoff, qw, probs_h, pat, use_pm, pull=None):
            T, g = h // 4, h % 4
            pm_t = [None]
            for mk in range(MT):
                if use_pm and mk % 4 == 2:
                    pm_t[0] = pmp.tile([P, N], F32, tag="pm", name=f"spm{h}_{mk}")
                if use_pm and mk % 4 >= 2:
                    sp = pm_t[0][:, ts(mk % 2, 512)][:, 0:qw]
                else:
                    spt = scp.tile([P, qw], F32, tag="sc", name=f"sp{h}_{mk}")
                    sp = spt[:]
                nc.tensor.matmul(
                    sp, kT[32 * g:32 * g + 32, T, :, ts(mk, P)],
                    qT[32 * g:32 * g + 32, T, :, qoff:qoff + qw],
                    start=True, stop=True, perf_mode=DR,
                    tile_position=(32 * g, 0),
                )
                emit_exp(pat[mk], probs_h[:, mk, :], sp)
                if pull is not None and mk % PULL_MOD == PULL_MOD - 1:
                    pull(1)

        def pav_head(h, qoff, qw, probs_h):
            mc_h = h // 2
            pv = pvp.tile([P, qw], F32, tag="pv", name="pv")
            for j in range(4):
                nc.tensor.matmul(
                    pv[0:HD + 1, :],
                    v_sb[:, 2 * j:2 * j + 2, h * VP:h * VP + HD + 1],
                    probs_h[:, 2 * j:2 * j + 2, :],
                    start=(j == 0), stop=(j == 3), perf_mode=DR,
                    skip_group_check=True,
                )
            srow = otmpp.tile([1, qw], F16, tag="srow", name="srow")
            with nc.allow_low_precision(reason="softmax denom ~1e3, fp16 ok"):
                nc.vector.reciprocal(srow[:], pv[HD:HD + 1, :])
            # broadcast 1/denom across 64 partitions (gpsimd custom op)
            rb = otmpp.tile([HD, qw], F16, tag="rb", name="rb")
            nc.gpsimd.partition_broadcast(rb[:], srow[:], channels=HD)
            if h % 2 == 0:
                nc.vector.tensor_mul(
                    oT[0:HD, mc_h, qoff:qoff + qw], pv[0:HD, :], rb[:]
                )
            else:
                o_t = otmpp.tile([HD, qw], F8, tag="o_t", name="o_t")
                nc.vector.tensor_mul(o_t[:], pv[0:HD, :], rb[:])
                nc.gpsimd.dma_start(oT[HD:P, mc_h, qoff:qoff + qw], o_t[:])

        oT = oTp.tile([P, KC, N], F8, tag="oT")
        x1 = x1p.tile([P, MT, D], F16)
        h2T = h2Tp.tile([P, KC, N], F8, tag="h2T")
        a1 = a1p.tile([P, 4, KC, N], F8)

        # ---- mlp generators (thunk steps pulled between score matmuls) ----
        proj_pieces = [None, None]

        def gen_proj(mt):
            pm = pmp.tile([P, N], F32, tag="pm", name=f"projpm{mt}")
            for kk in range(4):
                nc.tensor.matmul(
                    pm[:, 0:512], oT[:, 2 * kk:2 * kk + 2, ts(mt, P)],
                    proj_pieces[0][:, 2 * kk:2 * kk + 2, :],
                    start=(kk == 0), stop=(kk == 3), perf_mode=DR,
                )
            yield
            for kk in range(4):
                nc.tensor.matmul(
                    pm[:, 512:1024], oT[:, 2 * kk:2 * kk + 2, ts(mt, P)],
                    proj_pieces[1][:, 2 * kk:2 * kk + 2, :],
                    start=(kk == 0), stop=(kk == 3), perf_mode=DR,
                )
            for ph in range(2):
                nc.tensor.matmul(pm[:, ts(ph, 512)], identsw[:],
                                 x16[:, mt, ts(ph, 512)], start=False,
                                 stop=True, skip_group_check=True)
                if apply_c1:
                    nc.tensor.matmul(pm[:, ts(ph, 512)], ones16[0:1, :],
                                     c1row[:, ts(ph, 512)], start=False,
                                     stop=True, skip_group_check=True)
            yield
            eng_copy(nc, mt, x1[:, mt, :], pm[:])
            yield
            h = ln_stats(x1[:, mt, :], mt, h2T, ln2s, ln2b, ln2_triv)
            yield
            ln_transp(h, h2T, mt, ln2s, ln2b, ln2_triv, mt % 2)

        def gen_fc1(p8, qoff, qw):
            w1_t = w1p.tile([P, KC, 2, 512], F8, tag="w1", name=f"w1p{p8}")
            nc.sync.dma_start(
                w1_t[:].rearrange("p a b c -> p (a b c)"), wfc1_d.ap()[p8]
            )
            bp = min(4, N // qw)     # mh blocks packed per pm tile
            gelu_todo = None

            def emit_gelu(pm, pmi):
                mhg0 = p8 * 4 + pmi * bp
                if fc1b_triv:
                    nc.scalar.activation(
                        a1[:, mhg0 // 8, mhg0 % 8:mhg0 % 8 + bp, qoff:qoff + qw],
                        pm[:, 0:bp * qw].rearrange("p (a b) -> p a b", b=qw),
                        AF.Gelu_apprx_tanh, bias=0.0, scale=RSW,
                    )
                else:
                    for mi in range(bp):
                        mhg = mhg0 + mi
                        nc.scalar.activation(
                            a1[:, mhg // 8, mhg % 8, qoff:qoff + qw],
                            pm[:, mi * qw:(mi + 1) * qw],
                            AF.Gelu_apprx_tanh, bias=bfc1[:, mhg:mhg + 1],
                            scale=RSW,
                        )

            for pmi in range(4 // bp):
                pm = pmp.tile([P, N], F32, tag="pm", name=f"fc1pm{p8}_{pmi}")
                for mi in range(bp):
                    mh_l = pmi * bp + mi
                    for kc in range(KC):
                        nc.tensor.matmul(
                            pm[:, mi * qw:(mi + 1) * qw],
                            w1_t[:, kc, :, ts(mh_l, P)],
                            slot0(h2T[:, kc, qoff:qoff + qw], qw),
                            start=(kc == 0), stop=(kc == KC - 1), perf_mode=DR,
                        )
                yield
                emit_gelu(pm, pmi)

        def w2_piece(g, i):
            t = w2p.tile([P, KC, 2, N], F8, tag="w2", name=f"w2g{g}_{i}")
            nc.sync.dma_start(
                t[:].rearrange("p a b c -> p (a b c)"), wfc2_d.ap()[g]
            )
            return t

        def gen_fc2(mt, gpair, w2ab, final):
            pm = pmp.tile([P, N], F32, tag="pm", name=f"fc2pm{mt}_{gpair}")
            for ph in range(2):
                for gl in range(2):
                    g = gpair * 2 + gl
                    for kc in range(KC):
                        nc.tensor.matmul(
                            pm[:, ts(ph, 512)],
                            slot0(a1[:, g, kc, ts(mt, P)], P),
                            w2ab[gl][:, kc, :, ts(ph, 512)],
                            start=(gl == 0 and kc == 0),
                            stop=(gl == 1 and kc == KC - 1), perf_mode=DR,
                        )
                # x1 already holds SW*x1 (after pass 1, SW*(x1+mlp_g01))
                nc.tensor.matmul(pm[:, ts(ph, 512)], ident[:],
                                 x1[:, mt, ts(ph, 512)], start=False,
                                 stop=True, skip_group_check=True)
                if final and apply_bfc2:
                    nc.tensor.matmul(pm[:, ts(ph, 512)], ones16[0:1, :],
                                     b2row[:, ts(ph, 512)], start=False,
                                     stop=True, skip_group_check=True)
                if ph == 0:
                    yield
            yield
            if not final:
                eng_copy(nc, mt, x1[:, mt, :], pm[:])
            else:
                y16 = xload.tile([P, D], F16, tag="x_t", name="y16")
                scaled_copy(nc, mt, y16[:], pm[:], RSW)
                nc.sync.dma_start(y_d.ap()[ts(mt, P), :], y16[:])

        # ======= phases 3-5: attention chunks 512/256/256 + pipelined mlp ===
        from collections import deque
        probs_n = [0]

        def next_probs(qw):
            t = probs_alloc(probs_n[0], qw)
            probs_n[0] += 1
            return t

        # ---- phase 3: attention chunk A (queries 0:512) ----
        probs_q = []
        probs_q.append(next_probs(512))
        scores_head(0, 0, 512, probs_q[0], EXP_PAT_A, False)
        for mt in range(4):
            v_group(mt)
        probs_q.append(next_probs(512))
        scores_head(1, 0, 512, probs_q[1], EXP_PAT_A, False)
        for mt in range(4, 8):
            v_group(mt)
        for h in range(2, HEADS):
            probs_q.append(next_probs(512))
            scores_head(h, 0, 512, probs_q[-1], EXP_PAT_A, True)
            pav_head(h - 2, 0, 512, probs_q.pop(0))
        pav_head(HEADS - 2, 0, 512, probs_q.pop(0))
        pav_head(HEADS - 1, 0, 512, probs_q.pop(0))

        mark("attnA")
        # proj + fc2(g0,g1) weights land during attention A
        proj_pieces[0] = wpiece(wproj_v, 0)
        proj_pieces[1] = wpiece(wproj_v, 512)
        w2_01 = [w2_piece(0, 0), w2_piece(1, 0)]

        work = deque()

        def pull(k):
            for _ in range(k):
                while work:
                    try:
                        next(work[0])
                        break
                    except StopIteration:
                        work.popleft()

        def attn_chunk(qoff, qw, pat, late=None):
            probs_q = []
            for h in range(HEADS):
                probs_q.append(next_probs(qw))
                scores_head(h, qoff, qw, probs_q[-1], pat, h >= USE_PM_B, pull)
                if h >= 2:
                    pav_head(h - 2, qoff, qw, probs_q.pop(0))
                if late is not None and h == late[0]:
                    late[1]()
                pull(1)
            pav_head(HEADS - 2, qoff, qw, probs_q.pop(0))
            pav_head(HEADS - 1, qoff, qw, probs_q.pop(0))

        # ---- phase 4: attention B (512:1024) || mlp tokens 0:512 ----
        w2_23 = []
        for mt in range(4):
            work.append(gen_proj(mt))
        for p8 in range(8):
            work.append(gen_fc1(p8, 0, 512))
        for mt in range(4):
            work.append(gen_fc2(mt, 0, w2_01, final=False))

        def add_fc2b_t1():
            w2_23.extend([w2_piece(2, 0), w2_piece(3, 0)])
            for mt in range(4):
                work.append(gen_fc2(mt, 1, w2_23, final=True))

        attn_chunk(512, 512, EXP_PAT_B, late=(8, add_fc2b_t1))
        # prefetch phase-5 weights while attention B drains
        w2_01b = [w2_piece(0, 1), w2_piece(1, 1)]
        while work:
            pull(1)

        mark("phase4")
        # ---- phase 5: mlp tokens 512:1024 ----
        for mt in range(4, 6):
            for _ in gen_proj(mt):
                pass
        w2_23b = [w2_piece(2, 1), w2_piece(3, 1)]
        for mt in range(6, 8):
            for _ in gen_proj(mt):
                pass
        for p8 in range(8):
            for _ in gen_fc1(p8, 512, 512):
                pass
        for mt in range(4, 8):
            for _ in gen_fc2(mt, 0, w2_01b, final=False):
                pass
        for mt in range(4, 8):
            for _ in gen_fc2(mt, 1, w2_23b, final=True):
                pass
        mark("end")

    nc.compile()
    return nc


_cache = {}
_last_inmaps = None


def _get_nc(*key):
    if key not in _cache:
        _cache[key] = build_block(*key)
    return _cache[key]


def _f8(a):
    import ml_dtypes
    return np.clip(np.asarray(a, np.float32), -240.0, 240.0).astype(
        ml_dtypes.float8_e4m3
    )


def _perm_qk():
    perm = np.empty(D, np.int64)
    i = 0
    for T in range(4):
        for s in range(2):
            for g in range(4):
                for p in range(32):
                    perm[i] = (4 * T + g) * HD + 32 * s + p
                    i += 1
    return perm


def kernel(
    x, w_qkv, b_qkv, w_proj, b_proj, ln1_scale, ln1_bias,
    ln2_scale, ln2_bias, w_fc1, b_fc1, w_fc2, b_fc2,
):
    x = np.asarray(x, np.float32)
    B = x.shape[0]
    b_qkv = np.asarray(b_qkv, np.float32)
    b_v = b_qkv[2 * D:]
    c1 = b_v.astype(np.float64) @ np.asarray(w_proj, np.float64) + np.asarray(
        b_proj, np.float64
    )
    c1 = c1.astype(np.float32)
    bfc2 = np.asarray(b_fc2, np.float32)
    ln1_scale = np.asarray(ln1_scale, np.float32)
    ln1_bias = np.asarray(ln1_bias, np.float32)
    ln2_scale = np.asarray(ln2_scale, np.float32)
    ln2_bias = np.asarray(ln2_bias, np.float32)
    ln1_triv = bool(np.all(ln1_scale == 1) and np.all(ln1_bias == 0))
    ln2_triv = bool(np.all(ln2_scale == 1) and np.all(ln2_bias == 0))
    qk_triv = bool(np.all(b_qkv[:2 * D] == 0))
    apply_c1 = bool(np.any(c1 != 0))
    apply_bfc2 = bool(np.any(bfc2 != 0))
    fc1b_triv = bool(np.all(np.asarray(b_fc1, np.float32) == 0))

    nc = _get_nc(ln1_triv, ln2_triv, qk_triv, apply_c1, apply_bfc2, fc1b_triv)

    perm = _perm_qk()
    w_qkv = np.asarray(w_qkv, np.float32)
    wq = w_qkv[:, :D][:, perm]
    wk = w_qkv[:, D:2 * D][:, perm]
    wv = w_qkv[:, 2 * D:]
    wqkv_p = np.concatenate([wq, wk, wv], axis=1)

    w_fc1 = np.asarray(w_fc1, np.float32) * SW
    w1_hi = _f8(w_fc1)
    w1_lo = _f8(w_fc1 - w1_hi.astype(np.float32))
    # [D, 2, HID] -> [piece, p, (kc two col)]
    w1s = np.stack([w1_hi, w1_lo], axis=1)
    w1s = w1s.reshape(KC, P, 2, 8, 512).transpose(3, 1, 0, 2, 4)
    w1s = np.ascontiguousarray(w1s.reshape(8, P, KC * 2 * 512))
    w_fc2 = np.asarray(w_fc2, np.float32) * SW
    w2_hi = _f8(w_fc2)
    w2_lo = _f8(w_fc2 - w2_hi.astype(np.float32))
    # [HID, 2, D] -> [g, p, (kc two col)]
    w2s = np.stack([w2_hi, w2_lo], axis=1)
    w2s = w2s.reshape(4, KC, P, 2, N).transpose(0, 2, 1, 3, 4)
    w2s = np.ascontiguousarray(w2s.reshape(4, P, KC * 2 * N))

    base = {
        "w_qkv": _f8(wqkv_p * SW),
        "w_proj": _f8(np.asarray(w_proj, np.float32) * SW),
        "w_fc1": w1s,
        "w_fc2": w2s,
        "b_fc1": np.asarray(b_fc1, np.float32),
        "ln1_scale": ln1_scale,
        "ln1_bias": ln1_bias,
        "ln2_scale": ln2_scale,
        "ln2_bias": ln2_bias,
    }
    if not qk_triv:
        bqk = b_qkv[:2 * D][np.concatenate([perm, D + perm])] * SW
        base["b_qkc"] = np.ascontiguousarray(
            bqk.reshape(16, P).T.astype(np.float32)
        )
    if apply_c1:
        base["c1"] = (c1 * SW).astype(np.float16)
    if apply_bfc2:
        base["b_fc2c"] = (bfc2 * SW).astype(np.float16)

    in_maps = [dict(base, x=np.ascontiguousarray(x[i])) for i in range(B)]
    global _last_inmaps
    _last_inmaps = in_maps
    last_err = None
    for _attempt in range(3):
        try:
            res = run_bass_kernel_spmd(nc, in_maps, core_ids=list(range(B)))
            break
        except Exception as e:  # transient NRT/axon worker failures
            last_err = e
            import time as _time

            _time.sleep(2.0)
    else:
        raise last_err
    out = np.stack([res.results[i]["y"] for i in range(B)], axis=0)
    return np.ascontiguousarray(out.astype(np.float32))


# revision 5
# speedup vs baseline: 1.0202x; 1.0066x over previous
"""Trainium2 Bass kernel v2 for the dense transformer block (pre-LN MHA + MLP).

Data parallel across 8 cores (batch -> core). Per core [1024, 1024]:

- fp8e4 DoubleRow matmuls everywhere the contraction allows:
  qkv / proj (k-chunk pairs), scores (head-dim split 2x32 across slot dim,
  32-partition stationary), pav (key-chunk pairs), fc1/fc2 (weight hi+lo
  fp8 split in the slot dim, activation streamed to both slots with a
  0-stride AP).
- Weights scaled x1024 on host (fp8 subnormal escape); descale folded into
  epilogues; x1 kept scaled by 1024 (LN is scale-invariant) so the residual
  adds are PE identity-matmuls and the only descale is the final y copy.
- Softmax exp split across ACT (hw exp) and DVE/GPSIMD (Schraudolph bit
  trick, fine since softmax normalizes away small relative error).
- Attention + MLP processed in query halves so the second half's exp wall
  overlaps the first half's MLP matmuls.
"""

from contextlib import ExitStack

import numpy as np

import concourse.bass as bass
import concourse.tile as tile
from concourse import bacc, mybir
from concourse.bass import ts
from concourse.bass_utils import run_bass_kernel_spmd
from concourse.masks import make_identity

F32 = mybir.dt.float32
F16 = mybir.dt.float16
F8 = mybir.dt.float8e4
I32 = mybir.dt.int32
AF = mybir.ActivationFunctionType
ALU = mybir.AluOpType
DR = mybir.MatmulPerfMode.DoubleRow

P = 128
N = 1024         # tokens per core
D = 1024
KC = 8           # contraction chunks of 128 over D
HEADS = 16
HD = 64
HID = 4096
EPS = 1e-6
NT = 2
MT = 8
QH = 512         # query half
SCALE = HD ** -0.5
SW = 1024.0      # weight scale (fp8)
RSW = 1.0 / SW
VP = 65          # v_sb per-head pitch: 64 dims + ones col (mk step 16*65)

# Schraudolph exp: exp(SCALE*x) ~ bitcast_f32(int32(EA*x + EB))
EA = (2 ** 23) / np.log(2.0) * SCALE
EB = 127.0 * (2 ** 23) - 486411.0

# exp engine split per (mk index 0..7): A=ACT hw exp, D=DVE+GPSIMD split
EXP_PAT_A = ["D", "A", "A", "A", "D", "A", "A", "A"]  # phase A (no mlp rival)
EXP_PAT_B = ["D", "A", "A", "D", "A", "A", "D", "A"]  # phase B (gelu on ACT)
PULL_MOD = 4       # pull a thunk after every PULL_MOD-th mk in scores_head
USE_PM_B = 99      # attnB heads >= this borrow pm psum slots for scores
PROBS_BUFS = 2
XLOAD_BUFS = 2


def slot0(ap_, n):
    """[p, 2(0-stride), n] view of a [p, n] AP: stream same data to both
    DoubleRow slots."""
    return bass.AP(
        tensor=ap_.tensor, offset=ap_.offset,
        ap=[list(ap_.ap[0]), [0, 2], [ap_.ap[-1][0], n]],
    )


def scaled_copy(nc, ei, dst, src, s):
    """dst = src * s on DVE (even) or ACT (odd). GPSIMD cannot read PSUM."""
    if ei % 2 == 0:
        nc.vector.tensor_scalar_mul(dst, src, s)
    else:
        nc.scalar.activation(dst, src, AF.Copy, scale=s)


def eng_copy(nc, ei, dst, src):
    if ei % 2 == 0:
        nc.vector.tensor_copy(dst, src)
    else:
        nc.scalar.copy(dst, src)


def build_block(ln1_triv, ln2_triv, qk_triv, apply_c1, apply_bfc2,
                fc1b_triv=True, stop_after=None, phase_marks=None):
    nc = bacc.Bacc("TRN2", target_bir_lowering=False, debug=False, num_devices=8)

    class _Stop(Exception):
        pass

    def mark(name):
        if phase_marks is not None:
            phase_marks.append((name, len(nc.m.functions[0].blocks[0].instructions)))
        if stop_after is not None and name == stop_after:
            raise _Stop()

    x_d = nc.dram_tensor("x", [N, D], F32, kind="ExternalInput")
    wqkv_d = nc.dram_tensor("w_qkv", [D, 3 * D], F8, kind="ExternalInput")
    wproj_d = nc.dram_tensor("w_proj", [D, D], F8, kind="ExternalInput")
    # piece-contiguous: [piece, p, (kc two col)] so each stream is a 2D DMA
    wfc1_d = nc.dram_tensor("w_fc1", [8, P, KC * 2 * 512], F8, kind="ExternalInput")
    wfc2_d = nc.dram_tensor("w_fc2", [4, P, KC * 2 * N], F8, kind="ExternalInput")
    bfc1_d = nc.dram_tensor("b_fc1", [HID], F32, kind="ExternalInput")
    ln1s_d = nc.dram_tensor("ln1_scale", [D], F32, kind="ExternalInput")
    ln1b_d = nc.dram_tensor("ln1_bias", [D], F32, kind="ExternalInput")
    ln2s_d = nc.dram_tensor("ln2_scale", [D], F32, kind="ExternalInput")
    ln2b_d = nc.dram_tensor("ln2_bias", [D], F32, kind="ExternalInput")
    bqkc_d = (
        nc.dram_tensor("b_qkc", [P, 16], F32, kind="ExternalInput")
        if not qk_triv else None
    )
    c1_d = nc.dram_tensor("c1", [D], F16, kind="ExternalInput") if apply_c1 else None
    bfc2_d = (
        nc.dram_tensor("b_fc2c", [D], F16, kind="ExternalInput")
        if apply_bfc2 else None
    )
    y_d = nc.dram_tensor("y", [N, D], F16, kind="ExternalOutput")

    wqkv_v = wqkv_d.ap().rearrange("(kc p) n -> p kc n", p=P)
    wproj_v = wproj_d.ap().rearrange("(kc p) n -> p kc n", p=P)

    import contextlib
    with tile.TileContext(nc) as tc, ExitStack() as ctx, \
            contextlib.suppress(_Stop):
        ep = ctx.enter_context
        constp = ep(tc.tile_pool(name="const", bufs=1))
        xload = ep(tc.tile_pool(name="xload", bufs=XLOAD_BUFS))
        x16p = ep(tc.tile_pool(name="x16", bufs=1))
        htmpp = ep(tc.tile_pool(name="htmp", bufs=2))
        hTp = ep(tc.tile_pool(name="hT", bufs=1))
        qTp = ep(tc.tile_pool(name="qT", bufs=1))
        kTp = ep(tc.tile_pool(name="kT", bufs=1))
        vp = ep(tc.tile_pool(name="vv", bufs=1))
        oTp = ep(tc.tile_pool(name="oT", bufs=1))
        probsp = ep(tc.tile_pool(name="probs", bufs=PROBS_BUFS))
        x1p = ep(tc.tile_pool(name="x1", bufs=1))
        h2Tp = ep(tc.tile_pool(name="h2T", bufs=1))
        a1p = ep(tc.tile_pool(name="a1", bufs=1))
        wqp = ep(tc.tile_pool(name="wq", bufs=4))
        w1p = ep(tc.tile_pool(name="w1", bufs=2))
        w2p = ep(tc.tile_pool(name="w2", bufs=2))
        statsp = ep(tc.tile_pool(name="stats", bufs=4))
        otmpp = ep(tc.tile_pool(name="otmp", bufs=2))
        itp = ep(tc.tile_pool(name="it", bufs=2))
        pmp = ep(tc.tile_pool(name="pm", bufs=2, space="PSUM"))
        scp = ep(tc.tile_pool(name="sc", bufs=3, space="PSUM"))
        pvp = ep(tc.tile_pool(name="pv", bufs=1, space="PSUM"))

        # ---- first x tile load before anything ----
        x_t0 = xload.tile([P, D], F32, tag="x_t")
        nc.sync.dma_start(x_t0[:, 0:512], x_d.ap()[ts(0, P), 0:512])
        nc.sync.dma_start(x_t0[:, 512:1024], x_d.ap()[ts(0, P), 512:1024])

        # ---- constants ----
        ident = constp.tile([P, P], F16)
        make_identity(nc, ident[:])
        identsw = constp.tile([P, P], F16)
        make_identity(nc, identsw[:])
        nc.gpsimd.tensor_scalar_mul(identsw[:], identsw[:], SW)
        eps_t = constp.tile([P, 1], F32)
        nc.vector.memset(eps_t[:], EPS)
        ones16 = constp.tile([1, P], F16)
        nc.vector.memset(ones16[:], 1.0)
        ln1s = ln1b = ln2s = ln2b = None
        if not ln1_triv:
            ln1s = constp.tile([P, KC], F32)
            nc.gpsimd.dma_start(ln1s[:], ln1s_d.ap().rearrange("(k p) -> p k", p=P))
            ln1b = constp.tile([P, KC], F32)
            nc.gpsimd.dma_start(ln1b[:], ln1b_d.ap().rearrange("(k p) -> p k", p=P))
        if not ln2_triv:
            ln2s = constp.tile([P, KC], F32)
            nc.gpsimd.dma_start(ln2s[:], ln2s_d.ap().rearrange("(k p) -> p k", p=P))
            ln2b = constp.tile([P, KC], F32)
            nc.gpsimd.dma_start(ln2b[:], ln2b_d.ap().rearrange("(k p) -> p k", p=P))
        bqk = None
        if not qk_triv:
            bqk = constp.tile([P, 16], F32)
            nc.gpsimd.dma_start(bqk[:], bqkc_d.ap())
        bfc1 = constp.tile([P, HID // P], F32)
        nc.gpsimd.dma_start(bfc1[:], bfc1_d.ap().rearrange("(m p) -> p m", p=P))
        if apply_c1:
            c1row = constp.tile([1, D], F16)
            nc.gpsimd.dma_start(c1row[:], c1_d.ap().unsqueeze(0))
        if apply_bfc2:
            b2row = constp.tile([1, D], F16)
            nc.gpsimd.dma_start(b2row[:], bfc2_d.ap().unsqueeze(0))

        x16 = x16p.tile([P, MT, D], F16)
        hT = hTp.tile([P, KC, N], F8, tag="hT")

        def ln_stats(src_ap, mt, out_T, s_cols, b_cols, trivial):
            """LN stats + normalize into htmp; returns h staging tile."""
            st = statsp.tile([P, 2, 6], F32, tag="st")
            xr = src_ap.rearrange("p (a b) -> p a b", b=512)
            nc.vector.bn_stats(st[:, 0, :], xr[:, 0, :])
            nc.vector.bn_stats(st[:, 1, :], xr[:, 1, :])
            mv = statsp.tile([P, 2], F32, tag="mv")
            nc.vector.bn_aggr(mv[:], st[:])
            rstd = statsp.tile([P, 1], F32, tag="rstd")
            nc.scalar.activation(rstd[:], mv[:, 1:2], AF.Sqrt, bias=eps_t[:])
            nc.vector.reciprocal(rstd[:], rstd[:])
            h = htmpp.tile([P, D], F16, tag="h")
            nc.vector.tensor_scalar(
                out=h[:, 0:512], in0=src_ap[:, 0:512], scalar1=mv[:, 0:1],
                scalar2=rstd[:], op0=ALU.subtract, op1=ALU.mult,
            )
            nc.gpsimd.tensor_scalar(
                out=h[:, 512:1024], in0=src_ap[:, 512:1024], scalar1=mv[:, 0:1],
                scalar2=rstd[:], op0=ALU.subtract, op1=ALU.mult,
            )
            return h

        def ln_transp(h, out_T, mt, s_cols, b_cols, trivial, ei, kcs=None):
            """Transpose h chunks into out_T[:, kc, mt-window].

            Trivial path: all transposes land in ONE fp16 psum tile (1 bank)
            followed by a single merged copy."""
            kcs = kcs if kcs is not None else range(KC)
            if trivial:
                tp = scp.tile([P, KC, P], F16, tag="sc", name=f"tp{mt}")
                for kc in kcs:
                    nc.tensor.transpose(tp[:, kc, :], h[:, ts(kc, P)], ident[:])
                eng_copy(nc, ei, out_T[:, :, ts(mt, P)], tp[:])
            else:
                for kc in kcs:
                    pt_t = scp.tile([P, P], F16, tag="sc", name=f"pt{mt}_{kc}")
                    nc.tensor.transpose(pt_t[:], h[:, ts(kc, P)], ident[:])
                    nc.vector.tensor_scalar(
                        out=out_T[:, kc, ts(mt, P)], in0=pt_t[:],
                        scalar1=s_cols[:, kc:kc + 1], scalar2=b_cols[:, kc:kc + 1],
                        op0=ALU.mult, op1=ALU.add,
                    )

        # ---- phase 1: LN1 + transpose + x16 ----
        def ln1_mt(mt):
            if mt == 0:
                x_t = x_t0
            else:
                x_t = xload.tile([P, D], F32, tag="x_t", name=f"x_t{mt}")
                nc.sync.dma_start(x_t[:, 0:512], x_d.ap()[ts(mt, P), 0:512])
                nc.sync.dma_start(x_t[:, 512:1024], x_d.ap()[ts(mt, P), 512:1024])
            eng16 = nc.gpsimd if mt % 2 == 0 else nc.vector
            eng16.tensor_copy(x16[:, mt, :], x_t[:])
            h = ln_stats(x_t[:], mt, hT, ln1s, ln1b, ln1_triv)
            ln_transp(h, hT, mt, ln1s, ln1b, ln1_triv, mt % 2)

        for mt in range(4):
            ln1_mt(mt)

        # ---- phase 2: qkv (nt=0 overlaps LN1 of tiles 4..7) ----
        qT = qTp.tile([P, 4, 2, N], F8, tag="qT")
        kT = kTp.tile([P, 4, 2, N], F8, tag="kT")
        v_sb = vp.tile([P, MT, HEADS * VP], F8, tag="vv")

        def wpiece(view, n0, w=512):
            t = wqp.tile([P, KC, 512], F8, tag="w")
            nc.sync.dma_start(t[:, :, 0:w], view[:, :, n0:n0 + w])
            return t

        def qkv_group_nt(piece, col_l, dst, gi, nt):
            """One (T, s) q/k group for one token half -> dst fp8."""
            pm = pmp.tile([P, 512], F32, tag="pm", name=f"qk{gi}_{nt}")
            for kk in range(4):
                nc.tensor.matmul(
                    pm[:],
                    piece[:, 2 * kk:2 * kk + 2, ts(col_l, P)],
                    hT[:, 2 * kk:2 * kk + 2, ts(nt, 512)],
                    start=(kk == 0), stop=(kk == 3), perf_mode=DR,
                )
            if qk_triv:
                scaled_copy(nc, gi + nt, dst[:, ts(nt, 512)], pm[:], RSW)
            else:
                nc.vector.tensor_scalar(
                    out=dst[:, ts(nt, 512)], in0=pm[:],
                    scalar1=bqk[:, gi:gi + 1], scalar2=RSW,
                    op0=ALU.add, op1=ALU.mult,
                )

        qk_pieces = []
        for half in (1, 0):  # k first so scores can start early
            for piece_i in range(2):
                qk_pieces.append(wpiece(wqkv_v, half * 1024 + piece_i * 512))

        ln1_left = list(range(4, 8))
        gi = 0
        for nt in range(NT):
            for pi, piece in enumerate(qk_pieces):
                for lT in range(2):
                    for s in range(2):
                        half = 1 - pi // 2
                        T = (pi % 2) * 2 + lT
                        dst_t = kT if half == 1 else qT
                        qkv_group_nt(piece, lT * 2 + s, dst_t[:, T, s, :],
                                     (1 - half) * 8 + T * 2 + s, nt)
                if nt == 0 and ln1_left:
                    ln1_mt(ln1_left.pop(0))

        mark("ln1")
        mark("qkv_qk")
        v_pieces = [wpiece(wqkv_v, n0) for n0 in (2048, 2560)]

        def v_group(mt):
            v_row = v_sb[:, mt, :].rearrange("p (h c) -> p h c", c=VP)
            nc.vector.memset(v_row[:, :, HD:HD + 1], 1.0)
            pm = pmp.tile([P, N], F32, tag="pm")
            for vh in range(2):
                for kk in range(4):
                    nc.tensor.matmul(
                        pm[:, ts(vh, 512)],
                        hT[:, 2 * kk:2 * kk + 2, ts(mt, P)],
                        v_pieces[vh][:, 2 * kk:2 * kk + 2, :],
                        start=(kk == 0), stop=(kk == 3), perf_mode=DR,
                    )
            src = pm[:].rearrange("p (h c) -> p h c", c=HD)
            scaled_copy(nc, mt, v_row[:, :, 0:HD], src, RSW)

        # ---- attention machinery ----
        def probs_alloc(i, qw):
            if i % 3 == 2:
                return hTp.tile([P, KC, qw], F8, tag="hT", name=f"pb{i}")
            return probsp.tile([P, KC, qw], F8, tag="probs", name=f"pb{i}")

        def emit_exp(engc, dst, src_ps):
            if engc == "A":
                nc.scalar.activation(dst, src_ps, AF.Exp, scale=SCALE)
            else:
                it = itp.tile([P, src_ps.shape[-1]], I32, tag="it", name="it")
                nc.vector.tensor_scalar(
                    out=it[:], in0=src_ps, scalar1=EA, scalar2=EB,
                    op0=ALU.mult, op1=ALU.add,
                )
                nc.gpsimd.tensor_copy(dst, it[:].bitcast(F32))

        def scores_head(h, qoff, qw, probs_h, pat, use_pm, pull=None):
            T, g = h // 4, h % 4
            pm_t = [None]
            for mk in range(MT):
                if use_pm and mk % 4 == 2:
                    pm_t[0] = pmp.tile([P, N], F32, tag="pm", name=f"spm{h}_{mk}")
                if use_pm and mk % 4 >= 2:
                    sp = pm_t[0][:, ts(mk % 2, 512)][:, 0:qw]
                else:
                    spt = scp.tile([P, qw], F32, tag="sc", name=f"sp{h}_{mk}")
                    sp = spt[:]
                nc.tensor.matmul(
                    sp, kT[32 * g:32 * g + 32, T, :, ts(mk, P)],
                    qT[32 * g:32 * g + 32, T, :, qoff:qoff + qw],
                    start=True, stop=True, perf_mode=DR,
                    tile_position=(32 * g, 0),
                )
                emit_exp(pat[mk], probs_h[:, mk, :], sp)
                if pull is not None and mk % PULL_MOD == PULL_MOD - 1:
                    pull(1)

        def pav_head(h, qoff, qw, probs_h):
            mc_h = h // 2
            pv = pvp.tile([P, qw], F32, tag="pv", name="pv")
            for j in range(4):
                nc.tensor.matmul(
                    pv[0:HD + 1, :],
                    v_sb[:, 2 * j:2 * j + 2, h * VP:h * VP + HD + 1],
                    probs_h[:, 2 * j:2 * j + 2, :],
                    start=(j == 0), stop=(j == 3), perf_mode=DR,
                    skip_group_check=True,
                )
            srow = otmpp.tile([1, qw], F16, tag="srow", name="srow")
            with nc.allow_low_precision(reason="softmax denom ~1e3, fp16 ok"):
                nc.vector.reciprocal(srow[:], pv[HD:HD + 1, :])
            # broadcast 1/denom across 64 partitions (gpsimd custom op)
            rb = otmpp.tile([HD, qw], F16, tag="rb", name="rb")
            nc.gpsimd.partition_broadcast(rb[:], srow[:], channels=HD)
            if h % 2 == 0:
                nc.vector.tensor_mul(
                    oT[0:HD, mc_h, qoff:qoff + qw], pv[0:HD, :], rb[:]
                )
            else:
                o_t = otmpp.tile([HD, qw], F8, tag="o_t", name="o_t")
                nc.vector.tensor_mul(o_t[:], pv[0:HD, :], rb[:])
                nc.gpsimd.dma_start(oT[HD:P, mc_h, qoff:qoff + qw], o_t[:])

        oT = oTp.tile([P, KC, N], F8, tag="oT")
        x1 = x1p.tile([P, MT, D], F16)
        h2T = h2Tp.tile([P, KC, N], F8, tag="h2T")
        a1 = a1p.tile([P, 4, KC, N], F8)

        # ---- mlp generators (thunk steps pulled between score matmuls) ----
        proj_pieces = [None, None]

        def gen_proj(mt):
            pm = pmp.tile([P, N], F32, tag="pm", name=f"projpm{mt}")
            for kk in range(4):
                nc.tensor.matmul(
                    pm[:, 0:512], oT[:, 2 * kk:2 * kk + 2, ts(mt, P)],
                    proj_pieces[0][:, 2 * kk:2 * kk + 2, :],
                    start=(kk == 0), stop=(kk == 3), perf_mode=DR,
                )
            yield
            for kk in range(4):
                nc.tensor.matmul(
                    pm[:, 512:1024], oT[:, 2 * kk:2 * kk + 2, ts(mt, P)],
                    proj_pieces[1][:, 2 * kk:2 * kk + 2, :],
                    start=(kk == 0), stop=(kk == 3), perf_mode=DR,
                )
            for ph in range(2):
                nc.tensor.matmul(pm[:, ts(ph, 512)], identsw[:],
                                 x16[:, mt, ts(ph, 512)], start=False,
                                 stop=True, skip_group_check=True)
                if apply_c1:
                    nc.tensor.matmul(pm[:, ts(ph, 512)], ones16[0:1, :],
                                     c1row[:, ts(ph, 512)], start=False,
                                     stop=True, skip_group_check=True)
            yield
            eng_copy(nc, mt, x1[:, mt, :], pm[:])
            yield
            h = ln_stats(x1[:, mt, :], mt, h2T, ln2s, ln2b, ln2_triv)
            yield
            ln_transp(h, h2T, mt, ln2s, ln2b, ln2_triv, mt % 2)

        def gen_fc1(p8, qoff, qw):
            w1_t = w1p.tile([P, KC, 2, 512], F8, tag="w1", name=f"w1p{p8}")
            nc.sync.dma_start(
                w1_t[:].rearrange("p a b c -> p (a b c)"), wfc1_d.ap()[p8]
            )
            bp = min(4, N // qw)     # mh blocks packed per pm tile
            gelu_todo = None

            def emit_gelu(pm, pmi):
                mhg0 = p8 * 4 + pmi * bp
                if fc1b_triv:
                    nc.scalar.activation(
                        a1[:, mhg0 // 8, mhg0 % 8:mhg0 % 8 + bp, qoff:qoff + qw],
                        pm[:, 0:bp * qw].rearrange("p (a b) -> p a b", b=qw),
                        AF.Gelu_apprx_tanh, bias=0.0, scale=RSW,
                    )
                else:
                    for mi in range(bp):
                        mhg = mhg0 + mi
                        nc.scalar.activation(
                            a1[:, mhg // 8, mhg % 8, qoff:qoff + qw],
                            pm[:, mi * qw:(mi + 1) * qw],
                            AF.Gelu_apprx_tanh, bias=bfc1[:, mhg:mhg + 1],
                            scale=RSW,
                        )

            for pmi in range(4 // bp):
                pm = pmp.tile([P, N], F32, tag="pm", name=f"fc1pm{p8}_{pmi}")
                for mi in range(bp):
                    mh_l = pmi * bp + mi
                    for kc in range(KC):
                        nc.tensor.matmul(
                            pm[:, mi * qw:(mi + 1) * qw],
                            w1_t[:, kc, :, ts(mh_l, P)],
                            slot0(h2T[:, kc, qoff:qoff + qw], qw),
                            start=(kc == 0), stop=(kc == KC - 1), perf_mode=DR,
                        )
                yield
                emit_gelu(pm, pmi)

        def w2_piece(g, i):
            t = w2p.tile([P, KC, 2, N], F8, tag="w2", name=f"w2g{g}_{i}")
            nc.sync.dma_start(
                t[:].rearrange("p a b c -> p (a b c)"), wfc2_d.ap()[g]
            )
            return t

        def gen_fc2(mt, gpair, w2ab, final):
            pm = pmp.tile([P, N], F32, tag="pm", name=f"fc2pm{mt}_{gpair}")
            for ph in range(2):
                for gl in range(2):
                    g = gpair * 2 + gl
                    for kc in range(KC):
                        nc.tensor.matmul(
                            pm[:, ts(ph, 512)],
                            slot0(a1[:, g, kc, ts(mt, P)], P),
                            w2ab[gl][:, kc, :, ts(ph, 512)],
                            start=(gl == 0 and kc == 0),
                            stop=(gl == 1 and kc == KC - 1), perf_mode=DR,
                        )
                # x1 already holds SW*x1 (after pass 1, SW*(x1+mlp_g01))
                nc.tensor.matmul(pm[:, ts(ph, 512)], ident[:],
                                 x1[:, mt, ts(ph, 512)], start=False,
                                 stop=True, skip_group_check=True)
                if final and apply_bfc2:
                    nc.tensor.matmul(pm[:, ts(ph, 512)], ones16[0:1, :],
                                     b2row[:, ts(ph, 512)], start=False,
                                     stop=True, skip_group_check=True)
                if ph == 0:
                    yield
            yield
            if not final:
                eng_copy(nc, mt, x1[:, mt, :], pm[:])
            else:
                y16 = xload.tile([P, D], F16, tag="x_t", name="y16")
                scaled_copy(nc, mt, y16[:], pm[:], RSW)
                nc.sync.dma_start(y_d.ap()[ts(mt, P), :], y16[:])

        # ======= phases 3-5: attention chunks 512/256/256 + pipelined mlp ===
        from collections import deque
        probs_n = [0]

        def next_probs(qw):
            t = probs_alloc(probs_n[0], qw)
            probs_n[0] += 1
            return t

        # ---- phase 3: attention chunk A (queries 0:512) ----
        probs_q = []
        probs_q.append(next_probs(512))
        scores_head(0, 0, 512, probs_q[0], EXP_PAT_A, False)
        for mt in range(4):
            v_group(mt)
        probs_q.append(next_probs(512))
        scores_head(1, 0, 512, probs_q[1], EXP_PAT_A, False)
        for mt in range(4, 8):
            v_group(mt)
        for h in range(2, HEADS):
            probs_q.append(next_probs(512))
            scores_head(h, 0, 512, probs_q[-1], EXP_PAT_A, True)
            pav_head(h - 2, 0, 512, probs_q.pop(0))
        pav_head(HEADS - 2, 0, 512, probs_q.pop(0))
        pav_head(HEADS - 1, 0, 512, probs_q.pop(0))

        mark("attnA")
        # proj + fc2(g0,g1) weights land during attention A
        proj_pieces[0] = wpiece(wproj_v, 0)
        proj_pieces[1] = wpiece(wproj_v, 512)
        w2_01 = [w2_piece(0, 0), w2_piece(1, 0)]

        work = deque()

        def pull(k):
            for _ in range(k):
                while work:
                    try:
                        next(work[0])
                        break
                    except StopIteration:
                        work.popleft()

        def attn_chunk(qoff, qw, pat, late=None):
            probs_q = []
            for h in range(HEADS):
                probs_q.append(next_probs(qw))
                scores_head(h, qoff, qw, probs_q[-1], pat, h >= USE_PM_B, pull)
                if h >= 2:
                    pav_head(h - 2, qoff, qw, probs_q.pop(0))
                if late is not None and h == late[0]:
                    late[1]()
                pull(1)
            pav_head(HEADS - 2, qoff, qw, probs_q.pop(0))
            pav_head(HEADS - 1, qoff, qw, probs_q.pop(0))

        # ---- phase 4: attention B (512:1024) || mlp tokens 0:512 ----
        w2_23 = []
        for mt in range(4):
            work.append(gen_proj(mt))
        for p8 in range(8):
            work.append(gen_fc1(p8, 0, 512))
        for mt in range(4):
            work.append(gen_fc2(mt, 0, w2_01, final=False))

        def add_fc2b_t1():
            w2_23.extend([w2_piece(2, 0), w2_piece(3, 0)])
            for mt in range(4):
                work.append(gen_fc2(mt, 1, w2_23, final=True))

        attn_chunk(512, 512, EXP_PAT_B, late=(8, add_fc2b_t1))
        # prefetch phase-5 weights while attention B drains
        w2_01b = [w2_piece(0, 1), w2_piece(1, 1)]
        while work:
            pull(1)

        mark("phase4")
        # ---- phase 5: mlp tokens 512:1024 ----
        for mt in range(4, 6):
            for _ in gen_proj(mt):
                pass
        w2_23b = [w2_piece(2, 1), w2_piece(3, 1)]
        for mt in range(6, 8):
            for _ in gen_proj(mt):
                pass
        for p8 in range(8):
            for _ in gen_fc1(p8, 512, 512):
                pass
        for mt in range(4, 8):
            for _ in gen_fc2(mt, 0, w2_01b, final=False):
                pass
        for mt in range(4, 8):
            for _ in gen_fc2(mt, 1, w2_23b, final=True):
                pass
        mark("end")

    nc.compile()
    return nc


_cache = {}
_last_inmaps = None


def _get_nc(*key):
    if key not in _cache:
        _cache[key] = build_block(*key)
    return _cache[key]


def _f8(a):
    import ml_dtypes
    return np.clip(np.asarray(a, np.float32), -240.0, 240.0).astype(
        ml_dtypes.float8_e4m3
    )


def _perm_qk():
    perm = np.empty(D, np.int64)
    i = 0
    for T in range(4):
        for s in range(2):
            for g in range(4):
                for p in range(32):
                    perm[i] = (4 * T + g) * HD + 32 * s + p
                    i += 1
    return perm


def kernel(
    x, w_qkv, b_qkv, w_proj, b_proj, ln1_scale, ln1_bias,
    ln2_scale, ln2_bias, w_fc1, b_fc1, w_fc2, b_fc2,
):
    x = np.asarray(x, np.float32)
    B = x.shape[0]
    b_qkv = np.asarray(b_qkv, np.float32)
    b_v = b_qkv[2 * D:]
    c1 = b_v.astype(np.float64) @ np.asarray(w_proj, np.float64) + np.asarray(
        b_proj, np.float64
    )
    c1 = c1.astype(np.float32)
    bfc2 = np.asarray(b_fc2, np.float32)
    ln1_scale = np.asarray(ln1_scale, np.float32)
    ln1_bias = np.asarray(ln1_bias, np.float32)
    ln2_scale = np.asarray(ln2_scale, np.float32)
    ln2_bias = np.asarray(ln2_bias, np.float32)
    ln1_triv = bool(np.all(ln1_scale == 1) and np.all(ln1_bias == 0))
    ln2_triv = bool(np.all(ln2_scale == 1) and np.all(ln2_bias == 0))
    qk_triv = bool(np.all(b_qkv[:2 * D] == 0))
    apply_c1 = bool(np.any(c1 != 0))
    apply_bfc2 = bool(np.any(bfc2 != 0))
    fc1b_triv = bool(np.all(np.asarray(b_fc1, np.float32) == 0))

    nc = _get_nc(ln1_triv, ln2_triv, qk_triv, apply_c1, apply_bfc2, fc1b_triv)

    perm = _perm_qk()
    w_qkv = np.asarray(w_qkv, np.float32)
    wq = w_qkv[:, :D][:, perm]
    wk = w_qkv[:, D:2 * D][:, perm]
    wv = w_qkv[:, 2 * D:]
    wqkv_p = np.concatenate([wq, wk, wv], axis=1)

    w_fc1 = np.asarray(w_fc1, np.float32) * SW
    w1_hi = _f8(w_fc1)
    w1_lo = _f8(w_fc1 - w1_hi.astype(np.float32))
    # [D, 2, HID] -> [piece, p, (kc two col)]
    w1s = np.stack([w1_hi, w1_lo], axis=1)
    w1s = w1s.reshape(KC, P, 2, 8, 512).transpose(3, 1, 0, 2, 4)
    w1s = np.ascontiguousarray(w1s.reshape(8, P, KC * 2 * 512))
    w_fc2 = np.asarray(w_fc2, np.float32) * SW
    w2_hi = _f8(w_fc2)
    w2_lo = _f8(w_fc2 - w2_hi.astype(np.float32))
    # [HID, 2, D] -> [g, p, (kc two col)]
    w2s = np.stack([w2_hi, w2_lo], axis=1)
    w2s = w2s.reshape(4, KC, P, 2, N).transpose(0, 2, 1, 3, 4)
    w2s = np.ascontiguousarray(w2s.reshape(4, P, KC * 2 * N))

    base = {
        "w_qkv": _f8(wqkv_p * SW),
        "w_proj": _f8(np.asarray(w_proj, np.float32) * SW),
        "w_fc1": w1s,
        "w_fc2": w2s,
        "b_fc1": np.asarray(b_fc1, np.float32),
        "ln1_scale": ln1_scale,
        "ln1_bias": ln1_bias,
        "ln2_scale": ln2_scale,
        "ln2_bias": ln2_bias,
    }
    if not qk_triv:
        bqk = b_qkv[:2 * D][np.concatenate([perm, D + perm])] * SW
        base["b_qkc"] = np.ascontiguousarray(
            bqk.reshape(16, P).T.astype(np.float32)
        )
    if apply_c1:
        base["c1"] = (c1 * SW).astype(np.float16)
    if apply_bfc2:
        base["b_fc2c"] = (bfc2 * SW).astype(np.float16)

    in_maps = [dict(base, x=np.ascontiguousarray(x[i])) for i in range(B)]
    global _last_inmaps
    _last_inmaps = in_maps
    last_err = None
    for _attempt in range(3):
        try:
            res = run_bass_kernel_spmd(nc, in_maps, core_ids=list(range(B)))
            break
        except Exception as e:  # transient NRT/axon worker failures
            last_err = e
            import time as _time

            _time.sleep(2.0)
    else:
        raise last_err
    out = np.stack([res.results[i]["y"] for i in range(B)], axis=0)
    return np.ascontiguousarray(out.astype(np.float32))
